# revision 15
# baseline (speedup 1.0000x reference)
"""NonLocalBlock (single-head attention, N=HW=4096, d=128) on 8 trn2 cores.

Sharding: data-parallel over batch (B=8) — one batch element per NeuronCore.
Per core, the whole block runs out of SBUF:

  xf (256, 4096) -> theta_T = wt@xf + bt      (128, N)   [PE + DVE bias]
                    phi     = wp@xf + bp      (128, N)   [PE + DVE bias]
                    g0      = (wg@xf)^T       (N, 128)   [PE bf16; bf16 x
                                                          via SWDGE cast-DMA]
  S^T[m, n] = sum_i phi[i,m] * theta_T[i,n]   (keys m on partitions)
  expS = exp(S^T - 40)                         [ACT]
  sums[n] = sum_m expS[m, n]                   [DVE/GP partials + PE fold]
  yu[o, n] = sum_m g0[m,o] expS[m,n]           (unnormalized)
  o[c,n] = (wW @ yu)[c,n] / sums[n] + bW'[c]
  out = o + xf   — via DMA: xf pre-stored to out, o added on top with an
                   SWDGE accumulate-store (CCE per-element add), so the
                   residual costs no compute-engine time.

Softmax runs without a per-row max: scores are ~N(0, 128) with empirical
|S| < ~91, so exp(S - 40) (a global shift — softmax-invariant) stays
inside fp32 range. The normalization is commuted past the wW matmul
(divide after, per-column), so the PSUM->SBUF copy of yu does not wait
on the reciprocal and the next q's matmuls start immediately.

Engine budget per core: PE ~136us (S/y matmuls), ACT ~134us (exp only),
DVE ~120us (bias adds, sum partials, epilogue), GpSimd ~50us (4 sum
chunks per q + SWDGE descriptor generation). Matmuls use float32r
(fp22, 1 PE pass); attention probabilities and g are bf16.
"""

import numpy as np
from contextlib import ExitStack

import concourse.bass as bass
import concourse.mybir as mybir
import concourse.tile as tile
from concourse import bacc

P = 128          # partitions / inter channels
C = 256          # input channels
F32 = mybir.dt.float32
F32R = mybir.dt.float32r
AF = mybir.ActivationFunctionType
ALU = mybir.AluOpType
BF16 = mybir.dt.bfloat16
CSHIFT = 40.0    # global score shift before exp (softmax-invariant)

B_FULL = 8
H_FULL = 64
W_FULL = 64
N_FULL = H_FULL * W_FULL

def build_nc(N=N_FULL, NQ=1024):
    """Build the single-core Bass module (SPMD: same NEFF on all 8 cores)."""
    assert N % 512 == 0 and NQ % 512 == 0 and N % NQ == 0
    MC = N // P                   # number of 128-row key chunks
    NB = NQ // 512                # 512-wide matmul blocks per quarter
    NQn = N // NQ                 # query quarters
    NBLK = N // 512               # 512-col x blocks

    nc = bacc.Bacc("TRN2", target_bir_lowering=False, debug=False)

    x_d = nc.dram_tensor("x", [C, N], F32R, kind="ExternalInput").ap()
    xbf_d = nc.dram_tensor("xbf", [C, N], BF16, kind="ExternalInput").ap()
    # weights host-packed to partition-major [128, 2*128] so DMAs are
    # trivially contiguous (one descriptor per partition)
    wtT_d = nc.dram_tensor("wtT", [P, 2 * P], F32R, kind="ExternalInput").ap()
    wpT_d = nc.dram_tensor("wpT", [P, 2 * P], F32R, kind="ExternalInput").ap()
    wgT_d = nc.dram_tensor("wgT", [P, 2 * P], BF16, kind="ExternalInput").ap()
    wWT_d = nc.dram_tensor("wWT", [P, C], F32R, kind="ExternalInput").ap()
    bt_d = nc.dram_tensor("bt", [P, 1], F32, kind="ExternalInput").ap()
    bp_d = nc.dram_tensor("bp", [P, 1], F32, kind="ExternalInput").ap()
    bWp_d = nc.dram_tensor("bWp", [P, 2], F32, kind="ExternalInput").ap()
    out_d = nc.dram_tensor("out", [C, N], F32R, kind="ExternalOutput").ap()

    x_v = x_d.rearrange("(k p) n -> k p n", p=P)
    xbf_v = xbf_d.rearrange("(k p) n -> k p n", p=P)
    out_v = out_d.rearrange("(k p) n -> k p n", p=P)

    with tile.TileContext(nc) as tc, ExitStack() as ctx:
        const = ctx.enter_context(tc.tile_pool(name="const", bufs=1))
        big = ctx.enter_context(tc.tile_pool(name="big", bufs=1))
        work = ctx.enter_context(tc.tile_pool(name="work", bufs=3))
        ps = ctx.enter_context(tc.tile_pool(name="ps", bufs=2, space="PSUM"))
        psy = ctx.enter_context(tc.tile_pool(name="psy", bufs=1, space="PSUM"))

        # ---- constant loads ----
        wtT_sb = const.tile([P, 2, P], F32R, name="wtT_sb")
        wpT_sb = const.tile([P, 2, P], F32R, name="wpT_sb")
        wgT_sb = const.tile([P, 2, P], BF16, name="wgT_sb")
        wWT_sb = const.tile([P, C], F32R, name="wWT_sb")
        bt_sb = const.tile([P, 1], F32, name="bt_sb")
        bp_sb = const.tile([P, 1], F32, name="bp_sb")
        bWp_sb = const.tile([P, 2], F32, name="bWp_sb")
        ones_sb = const.tile([P, P], BF16, name="ones_sb")
        cshift_sb = const.tile([P, 1], F32, name="cshift_sb")
        nc.vector.memset(cshift_sb[:], -CSHIFT)

        nc.sync.dma_start(wtT_sb[:], wtT_d.rearrange("p (k i) -> p k i", k=2))
        nc.sync.dma_start(wpT_sb[:], wpT_d.rearrange("p (k i) -> p k i", k=2))
        nc.sync.dma_start(wgT_sb[:], wgT_d.rearrange("p (k i) -> p k i", k=2))
        nc.sync.dma_start(wWT_sb[:], wWT_d)
        nc.sync.dma_start(bt_sb[:], bt_d)
        nc.sync.dma_start(bp_sb[:], bp_d)
        nc.sync.dma_start(bWp_sb[:], bWp_d)
        nc.vector.memset(ones_sb[:], 1.0)


        x_sb = big.tile([P, 2, N], F32R, name="x_sb")
        xbf_sb = big.tile([P, 2, N], BF16, name="xbf_sb")
        th_sb = big.tile([P, N], F32R, name="th_sb")   # theta^T (i, n)
        ph_sb = big.tile([P, N], F32R, name="ph_sb")   # phi (i, m)
        g_sb = big.tile([P, MC, P], BF16, name="g_sb")  # g0 (m_in, m_chunk, o)

        # ---- x load interleaved with th/ph projections, block by block.
        # xbf loads and the g matmuls are deferred behind the whole x
        # stream: x completes ~25% sooner (per-queue DMA bandwidth is the
        # startup limit), the th/ph proj-slot rotation never waits on
        # xbf, and g chunks still land well ahead of their y-matmul
        # consumers (one chunk per ~1.1us exp step).
        for blk in range(NBLK):
            sl = slice(blk * 512, (blk + 1) * 512)
            if blk < 2:
                # first blocks in 128-col pieces: per-queue DMA runs at
                # ~25 GB/s, so fine pieces across queues cut the latency
                # to the first theta/phi matmuls by several us
                for piece in range(4):
                    pl = slice(blk * 512 + piece * 128,
                               blk * 512 + (piece + 1) * 128)
                    for k in range(2):
                        nc.sync.dma_start(x_sb[:, k, pl], x_v[k, :, pl])
            else:
                for k in range(2):
                    nc.sync.dma_start(x_sb[:, k, sl], x_v[k, :, sl])

            th_ps = ps.tile([P, 512], F32, tag="proj", name="th_ps")
            nc.tensor.matmul(th_ps[:], wtT_sb[:, 0], x_sb[:, 0, sl],
                             start=True, stop=False)
            nc.tensor.matmul(th_ps[:], wtT_sb[:, 1], x_sb[:, 1, sl],
                             start=False, stop=True)
            nc.vector.tensor_scalar_add(th_sb[:, sl], th_ps[:], bt_sb[:, 0:1])

            ph_ps = ps.tile([P, 512], F32, tag="proj", name="ph_ps")
            nc.tensor.matmul(ph_ps[:], wpT_sb[:, 0], x_sb[:, 0, sl],
                             start=True, stop=False)
            nc.tensor.matmul(ph_ps[:], wpT_sb[:, 1], x_sb[:, 1, sl],
                             start=False, stop=True)
            nc.vector.tensor_scalar_add(ph_sb[:, sl], ph_ps[:], bp_sb[:, 0:1])

        for blk in range(NBLK):
            sl = slice(blk * 512, (blk + 1) * 512)
            for k in range(2):
                nc.sync.dma_start(xbf_sb[:, k, sl], xbf_v[k, :, sl])
            # g chunks for this block (bf16: full-rate 128-col matmuls)
            for j in range(4):
                mc = blk * 4 + j
                msl = slice(mc * P, (mc + 1) * P)
                g_ps = ps.tile([P, P], F32, tag="proj", name="g_ps")
                nc.tensor.matmul(g_ps[:], xbf_sb[:, 0, msl], wgT_sb[:, 0],
                                 start=True, stop=False)
                nc.tensor.matmul(g_ps[:], xbf_sb[:, 1, msl], wgT_sb[:, 1],
                                 start=False, stop=True)
                nc.vector.tensor_copy(g_sb[:, mc], g_ps[:])

        # residual: pre-store x to out for the first 3 quarters (their o
        # is added on top with SWDGE accumulate-stores); the last quarter
        # adds x on DVE and does a plain store to shorten the tail
        NPRE = N - NQ
        for k in range(2):
            nc.sync.dma_start(out_v[k, :, 0:NPRE], x_sb[:, k, 0:NPRE])

        # ---- attention main loop ----
        for q in range(NQn):
            qsl = slice(q * NQ, (q + 1) * NQ)
            y_ps = psy.tile([P, NQ], F32, tag="y", name="y_ps")
            acc = None             # DVE accumulator

            for mc in range(MC):
                msl = slice(mc * P, (mc + 1) * P)
                s_ps = ps.tile([P, NQ], F32, tag="s", name="s_ps")
                for b in range(NB):
                    bsl = slice(b * 512, (b + 1) * 512)
                    nc.tensor.matmul(
                        s_ps[:, bsl], ph_sb[:, msl],
                        th_sb[:, q * NQ + b * 512: q * NQ + (b + 1) * 512],
                        start=True, stop=True)
                exp_sb = work.tile([P, NQ], BF16, tag="exp", bufs=8,
                                   name="exp_sb")
                nc.scalar.activation(exp_sb[:], s_ps[:], AF.Exp,
                                     bias=cshift_sb[:, 0:1])

                for b in range(NB):
                    bsl = slice(b * 512, (b + 1) * 512)
                    nc.tensor.matmul(
                        y_ps[:, bsl], g_sb[:, mc], exp_sb[:, bsl],
                        start=(mc == 0), stop=(mc == MC - 1),
                        skip_group_check=True)

                # column-sum partials on DVE, single accumulator
                # (tensor_tensor runs at 2x on bf16; GpSimd stays idle —
                # it shares an SBUF port with DVE and contention costs
                # more than its offload saves; the 692ns add keeps pace
                # with the 1114ns exp so the chain never falls behind)
                if acc is None:
                    acc = work.tile([P, NQ], BF16, tag="acc", bufs=1,
                                    name="acc_sb")
                    nc.vector.tensor_copy(acc[:], exp_sb[:])
                else:
                    nc.vector.tensor_add(acc[:], acc[:], exp_sb[:])

            # unnormalized y out of PSUM immediately (frees y_ps for q+1;
            # does NOT wait on the sum/reciprocal path)
            yt_sb = work.tile([P, NQ], F32R, tag="yt", bufs=2, name="yt_sb")
            nc.vector.tensor_copy(yt_sb[:], y_ps[:])

            # Whole epilogue runs on 1-bank "proj" PSUM tiles (idle during
            # attention) in 512-col halves, so neither the "s" slots (S
            # prefetch) nor the "y" slot (next q's accumulation) is ever
            # held by epilogue work.
            recip_sb = work.tile([P, NQ], F32, tag="recip", bufs=2,
                                 name="recip_sb")
            for b in range(NB):
                bsl = slice(b * 512, (b + 1) * 512)
                sum_ps = ps.tile([P, 512], F32, tag="proj", name="sum_ps")
                nc.tensor.matmul(sum_ps[:], ones_sb[:], acc[:, bsl],
                                 start=True, stop=True,
                                 skip_group_check=True)
                nc.vector.reciprocal_approx_fast(recip_sb[:, bsl], sum_ps[:])

            # o = (wW @ yu) * recip + bW'; the +x rides the accumulate-
            # store except on the last q, where a DVE add + plain store in
            # 512-col pieces shortens the kernel tail.
            last = (q == NQn - 1)
            for h in range(2):
                o_sb = work.tile([P, NQ], F32R, tag="o", bufs=4, name="o_sb")
                for b in range(NB):
                    bsl = slice(b * 512, (b + 1) * 512)
                    wy_ps = ps.tile([P, 512], F32, tag="proj", name="wy_ps")
                    nc.tensor.matmul(
                        wy_ps[:], wWT_sb[:, h * P:(h + 1) * P],
                        yt_sb[:, bsl], start=True, stop=True)
                    nc.vector.tensor_mul(o_sb[:, bsl], wy_ps[:],
                                         recip_sb[:, bsl])
                    nc.vector.tensor_scalar_add(o_sb[:, bsl], o_sb[:, bsl],
                                                bWp_sb[:, h:h + 1])
                    if last:
                        nc.vector.tensor_add(o_sb[:, bsl], o_sb[:, bsl],
                                             x_sb[:, h, q * NQ + b * 512:
                                                  q * NQ + (b + 1) * 512])
                        nc.sync.dma_start(
                            out_v[h, :, q * NQ + b * 512:
                                  q * NQ + (b + 1) * 512], o_sb[:, bsl])
                if not last:
                    nc.gpsimd.dma_start(out_v[h, :, qsl], o_sb[:],
                                        accum_op=ALU.add)

    nc.compile()
    return nc


_CACHE = {}


def _built(key=(N_FULL, 1024)):
    if key not in _CACHE:
        _CACHE[key] = build_nc(*key)
    return _CACHE[key]


def make_in_maps(x, wg, bg, wt, bt, wp, bp, wW, bW):
    """Host-side prep: per-core input dicts (core b <- batch b)."""
    x = np.asarray(x, np.float32)
    B, C_, H, W = x.shape
    N = H * W
    xf = np.ascontiguousarray(x.reshape(B, C_, N))
    wg, bg, wt, bt, wp, bp, wW, bW = [
        np.asarray(a, np.float32) for a in (wg, bg, wt, bt, wp, bp, wW, bW)]
    def pack(w, dt=np.float32):  # (128, C) conv weight -> partition-major lhsT
        return np.ascontiguousarray(
            w.T.reshape(2, P, P).transpose(1, 0, 2).reshape(P, 2 * P)
        ).astype(dt)

    import ml_dtypes
    wtT, wpT = pack(wt), pack(wp)
    wgT = pack(wg, ml_dtypes.bfloat16)
    wWT = np.ascontiguousarray(wW.T)                       # (128, 256)
    bWp = (wW @ bg + bW).astype(np.float32)                # fold bg into bW
    bWp = np.ascontiguousarray(bWp.reshape(2, P).T)        # (128, 2)
    shared = {
        "wtT": wtT, "wpT": wpT, "wgT": wgT, "wWT": wWT,
        "bt": bt.reshape(P, 1).copy(), "bp": bp.reshape(P, 1).copy(),
        "bWp": bWp,
    }
    return [{"x": np.ascontiguousarray(xf[b]),
             "xbf": np.ascontiguousarray(xf[b].astype(ml_dtypes.bfloat16)),
             **shared} for b in range(B)]


def kernel(x, wg, bg, wt, bt, wp, bp, wW, bW):
    from concourse.bass_utils import run_bass_kernel_spmd

    B, C_, H, W = np.asarray(x).shape
    in_maps = make_in_maps(x, wg, bg, wt, bt, wp, bp, wW, bW)
    nc = _built()
    res = run_bass_kernel_spmd(nc, in_maps, core_ids=list(range(B)))
    out = np.stack([res.results[b]["out"] for b in range(B)])
    return out.reshape(B, C_, H, W).astype(np.float32)


# revision 16
# speedup vs baseline: 1.0458x; 1.0458x over previous
"""NonLocalBlock (single-head attention, N=HW=4096, d=128) on 8 trn2 cores.

Sharding: data-parallel over batch (B=8) — one batch element per NeuronCore.
Per core, the whole block runs out of SBUF:

  xf (256, 4096) -> theta_T = wt@xf + bt      (128, N)   [PE + DVE bias]
                    phi     = wp@xf + bp      (128, N)   [PE + DVE bias]
                    g0      = (wg@xf)^T       (N, 128)   [PE bf16; bf16 x
                                                          via SWDGE cast-DMA]
  S^T[m, n] = sum_i phi[i,m] * theta_T[i,n]   (keys m on partitions)
  expS = exp(S^T - 40)                         [ACT]
  sums[n] = sum_m expS[m, n]                   [DVE/GP partials + PE fold]
  yu[o, n] = sum_m g0[m,o] expS[m,n]           (unnormalized)
  o[c,n] = (wW @ yu)[c,n] / sums[n] + bW'[c]
  out = o + xf   — via DMA: xf pre-stored to out, o added on top with an
                   SWDGE accumulate-store (CCE per-element add), so the
                   residual costs no compute-engine time.

Softmax runs without a per-row max: scores are ~N(0, 128) with empirical
|S| < ~91, so exp(S - 40) (a global shift — softmax-invariant) stays
inside fp32 range. The normalization is commuted past the wW matmul
(divide after, per-column), so the PSUM->SBUF copy of yu does not wait
on the reciprocal and the next q's matmuls start immediately.

Engine budget per core: PE ~136us (S/y matmuls), ACT ~134us (exp only),
DVE ~120us (bias adds, sum partials, epilogue), GpSimd ~50us (4 sum
chunks per q + SWDGE descriptor generation). Matmuls use float32r
(fp22, 1 PE pass); attention probabilities and g are bf16.
"""

import numpy as np
from contextlib import ExitStack

import concourse.bass as bass
import concourse.mybir as mybir
import concourse.tile as tile
from concourse import bacc

P = 128          # partitions / inter channels
C = 256          # input channels
F32 = mybir.dt.float32
F32R = mybir.dt.float32r
AF = mybir.ActivationFunctionType
ALU = mybir.AluOpType
BF16 = mybir.dt.bfloat16
CSHIFT = 40.0    # global score shift before exp (softmax-invariant)

B_FULL = 8
H_FULL = 64
W_FULL = 64
N_FULL = H_FULL * W_FULL

def build_nc(N=N_FULL, NQ=1024):
    """Build the single-core Bass module (SPMD: same NEFF on all 8 cores)."""
    assert N % 512 == 0 and NQ % 512 == 0 and N % NQ == 0
    MC = N // P                   # number of 128-row key chunks
    NB = NQ // 512                # 512-wide matmul blocks per quarter
    NQn = N // NQ                 # query quarters
    NBLK = N // 512               # 512-col x blocks

    nc = bacc.Bacc("TRN2", target_bir_lowering=False, debug=False)

    x_d = nc.dram_tensor("x", [C, N], F32R, kind="ExternalInput").ap()
    xbf_d = nc.dram_tensor("xbf", [C, N], BF16, kind="ExternalInput").ap()
    # weights host-packed to partition-major [128, 2*128] so DMAs are
    # trivially contiguous (one descriptor per partition)
    wtT_d = nc.dram_tensor("wtT", [P, 2 * P], F32R, kind="ExternalInput").ap()
    wpT_d = nc.dram_tensor("wpT", [P, 2 * P], F32R, kind="ExternalInput").ap()
    wgT_d = nc.dram_tensor("wgT", [P, 2 * P], BF16, kind="ExternalInput").ap()
    wWT_d = nc.dram_tensor("wWT", [P, C], F32R, kind="ExternalInput").ap()
    bt_d = nc.dram_tensor("bt", [P, 1], F32, kind="ExternalInput").ap()
    bp_d = nc.dram_tensor("bp", [P, 1], F32, kind="ExternalInput").ap()
    bWp_d = nc.dram_tensor("bWp", [P, 2], F32, kind="ExternalInput").ap()
    out_d = nc.dram_tensor("out", [C, N], F32R, kind="ExternalOutput").ap()

    x_v = x_d.rearrange("(k p) n -> k p n", p=P)
    xbf_v = xbf_d.rearrange("(k p) n -> k p n", p=P)
    out_v = out_d.rearrange("(k p) n -> k p n", p=P)

    with tile.TileContext(nc) as tc, ExitStack() as ctx:
        const = ctx.enter_context(tc.tile_pool(name="const", bufs=1))
        big = ctx.enter_context(tc.tile_pool(name="big", bufs=1))
        work = ctx.enter_context(tc.tile_pool(name="work", bufs=3))
        ps = ctx.enter_context(tc.tile_pool(name="ps", bufs=2, space="PSUM"))
        psy = ctx.enter_context(tc.tile_pool(name="psy", bufs=1, space="PSUM"))

        # ---- constant loads ----
        wtT_sb = const.tile([P, 2, P], F32R, name="wtT_sb")
        wpT_sb = const.tile([P, 2, P], F32R, name="wpT_sb")
        wgT_sb = const.tile([P, 2, P], BF16, name="wgT_sb")
        wWT_sb = const.tile([P, C], F32R, name="wWT_sb")
        bt_sb = const.tile([P, 1], F32, name="bt_sb")
        bp_sb = const.tile([P, 1], F32, name="bp_sb")
        bWp_sb = const.tile([P, 2], F32, name="bWp_sb")
        ones_sb = const.tile([P, P], BF16, name="ones_sb")
        cshift_sb = const.tile([P, 1], F32, name="cshift_sb")
        nc.vector.memset(cshift_sb[:], -CSHIFT)

        nc.sync.dma_start(wtT_sb[:], wtT_d.rearrange("p (k i) -> p k i", k=2))
        nc.sync.dma_start(wpT_sb[:], wpT_d.rearrange("p (k i) -> p k i", k=2))
        nc.sync.dma_start(wgT_sb[:], wgT_d.rearrange("p (k i) -> p k i", k=2))
        nc.sync.dma_start(wWT_sb[:], wWT_d)
        nc.sync.dma_start(bt_sb[:], bt_d)
        nc.sync.dma_start(bp_sb[:], bp_d)
        nc.sync.dma_start(bWp_sb[:], bWp_d)
        nc.vector.memset(ones_sb[:], 1.0)


        x_sb = big.tile([P, 2, N], F32R, name="x_sb")
        xbf_sb = big.tile([P, 2, N], BF16, name="xbf_sb")
        th_sb = big.tile([P, N], F32R, name="th_sb")   # theta^T (i, n)
        ph_sb = big.tile([P, N], F32R, name="ph_sb")   # phi (i, m)
        g_sb = big.tile([P, MC, P], BF16, name="g_sb")  # g0 (m_in, m_chunk, o)

        # ---- x load interleaved with th/ph projections, block by block.
        # xbf loads and the g matmuls are deferred behind the whole x
        # stream: x completes ~25% sooner (per-queue DMA bandwidth is the
        # startup limit), the th/ph proj-slot rotation never waits on
        # xbf, and g chunks still land well ahead of their y-matmul
        # consumers (one chunk per ~1.1us exp step).
        for blk in range(NBLK):
            sl = slice(blk * 512, (blk + 1) * 512)
            for k in range(2):
                nc.sync.dma_start(x_sb[:, k, sl], x_v[k, :, sl])

            th_ps = ps.tile([P, 512], F32, tag="proj", name="th_ps")
            nc.tensor.matmul(th_ps[:], wtT_sb[:, 0], x_sb[:, 0, sl],
                             start=True, stop=False)
            nc.tensor.matmul(th_ps[:], wtT_sb[:, 1], x_sb[:, 1, sl],
                             start=False, stop=True)
            nc.vector.tensor_scalar_add(th_sb[:, sl], th_ps[:], bt_sb[:, 0:1])

            ph_ps = ps.tile([P, 512], F32, tag="proj", name="ph_ps")
            nc.tensor.matmul(ph_ps[:], wpT_sb[:, 0], x_sb[:, 0, sl],
                             start=True, stop=False)
            nc.tensor.matmul(ph_ps[:], wpT_sb[:, 1], x_sb[:, 1, sl],
                             start=False, stop=True)
            nc.vector.tensor_scalar_add(ph_sb[:, sl], ph_ps[:], bp_sb[:, 0:1])

        for blk in range(NBLK):
            sl = slice(blk * 512, (blk + 1) * 512)
            for k in range(2):
                nc.sync.dma_start(xbf_sb[:, k, sl], xbf_v[k, :, sl])
            # g chunks for this block (bf16: full-rate 128-col matmuls)
            for j in range(4):
                mc = blk * 4 + j
                msl = slice(mc * P, (mc + 1) * P)
                g_ps = ps.tile([P, P], F32, tag="proj", name="g_ps")
                nc.tensor.matmul(g_ps[:], xbf_sb[:, 0, msl], wgT_sb[:, 0],
                                 start=True, stop=False)
                nc.tensor.matmul(g_ps[:], xbf_sb[:, 1, msl], wgT_sb[:, 1],
                                 start=False, stop=True)
                nc.vector.tensor_copy(g_sb[:, mc], g_ps[:])

        # residual: pre-store x to out for the first 3 quarters (their o
        # is added on top with SWDGE accumulate-stores); the last quarter
        # adds x on DVE and does a plain store to shorten the tail
        NPRE = N - NQ
        for k in range(2):
            nc.sync.dma_start(out_v[k, :, 0:NPRE], x_sb[:, k, 0:NPRE])

        # ---- attention main loop ----
        for q in range(NQn):
            qsl = slice(q * NQ, (q + 1) * NQ)
            y_ps = psy.tile([P, NQ], F32, tag="y", name="y_ps")
            acc = None             # DVE accumulator

            for mc in range(MC):
                msl = slice(mc * P, (mc + 1) * P)
                s_ps = ps.tile([P, NQ], F32, tag="s", name="s_ps")
                for b in range(NB):
                    bsl = slice(b * 512, (b + 1) * 512)
                    nc.tensor.matmul(
                        s_ps[:, bsl], ph_sb[:, msl],
                        th_sb[:, q * NQ + b * 512: q * NQ + (b + 1) * 512],
                        start=True, stop=True)
                exp_sb = work.tile([P, NQ], BF16, tag="exp", bufs=8,
                                   name="exp_sb")
                nc.scalar.activation(exp_sb[:], s_ps[:], AF.Exp,
                                     bias=cshift_sb[:, 0:1])

                for b in range(NB):
                    bsl = slice(b * 512, (b + 1) * 512)
                    nc.tensor.matmul(
                        y_ps[:, bsl], g_sb[:, mc], exp_sb[:, bsl],
                        start=(mc == 0), stop=(mc == MC - 1),
                        skip_group_check=True)

                # column-sum partials on DVE, single accumulator
                # (tensor_tensor runs at 2x on bf16; GpSimd stays idle —
                # it shares an SBUF port with DVE and contention costs
                # more than its offload saves; the 692ns add keeps pace
                # with the 1114ns exp so the chain never falls behind)
                if acc is None:
                    acc = work.tile([P, NQ], BF16, tag="acc", bufs=1,
                                    name="acc_sb")
                    nc.vector.tensor_copy(acc[:], exp_sb[:])
                else:
                    nc.vector.tensor_add(acc[:], acc[:], exp_sb[:])

            # unnormalized y out of PSUM immediately (frees y_ps for q+1;
            # does NOT wait on the sum/reciprocal path)
            yt_sb = work.tile([P, NQ], F32R, tag="yt", bufs=2, name="yt_sb")
            nc.vector.tensor_copy(yt_sb[:], y_ps[:])

            # Whole epilogue runs on 1-bank "proj" PSUM tiles (idle during
            # attention) in 512-col halves, so neither the "s" slots (S
            # prefetch) nor the "y" slot (next q's accumulation) is ever
            # held by epilogue work.
            recip_sb = work.tile([P, NQ], F32, tag="recip", bufs=2,
                                 name="recip_sb")
            for b in range(NB):
                bsl = slice(b * 512, (b + 1) * 512)
                sum_ps = ps.tile([P, 512], F32, tag="proj", name="sum_ps")
                nc.tensor.matmul(sum_ps[:], ones_sb[:], acc[:, bsl],
                                 start=True, stop=True,
                                 skip_group_check=True)
                nc.vector.reciprocal_approx_fast(recip_sb[:, bsl], sum_ps[:])

            # o = (wW @ yu) * recip + bW'; the +x rides the accumulate-
            # store except on the last q, where a DVE add + plain store in
            # 512-col pieces shortens the kernel tail.
            last = (q == NQn - 1)
            for h in range(2):
                o_sb = work.tile([P, NQ], F32R, tag="o", bufs=4, name="o_sb")
                for b in range(NB):
                    bsl = slice(b * 512, (b + 1) * 512)
                    wy_ps = ps.tile([P, 512], F32, tag="proj", name="wy_ps")
                    nc.tensor.matmul(
                        wy_ps[:], wWT_sb[:, h * P:(h + 1) * P],
                        yt_sb[:, bsl], start=True, stop=True)
                    nc.vector.tensor_mul(o_sb[:, bsl], wy_ps[:],
                                         recip_sb[:, bsl])
                    nc.vector.tensor_scalar_add(o_sb[:, bsl], o_sb[:, bsl],
                                                bWp_sb[:, h:h + 1])
                    if last:
                        nc.vector.tensor_add(o_sb[:, bsl], o_sb[:, bsl],
                                             x_sb[:, h, q * NQ + b * 512:
                                                  q * NQ + (b + 1) * 512])
                        nc.sync.dma_start(
                            out_v[h, :, q * NQ + b * 512:
                                  q * NQ + (b + 1) * 512], o_sb[:, bsl])
                if not last:
                    nc.gpsimd.dma_start(out_v[h, :, qsl], o_sb[:],
                                        accum_op=ALU.add)

    nc.compile()
    return nc


_CACHE = {}


def _built(key=(N_FULL, 1024)):
    if key not in _CACHE:
        _CACHE[key] = build_nc(*key)
    return _CACHE[key]


def make_in_maps(x, wg, bg, wt, bt, wp, bp, wW, bW):
    """Host-side prep: per-core input dicts (core b <- batch b)."""
    x = np.asarray(x, np.float32)
    B, C_, H, W = x.shape
    N = H * W
    xf = np.ascontiguousarray(x.reshape(B, C_, N))
    wg, bg, wt, bt, wp, bp, wW, bW = [
        np.asarray(a, np.float32) for a in (wg, bg, wt, bt, wp, bp, wW, bW)]
    def pack(w, dt=np.float32):  # (128, C) conv weight -> partition-major lhsT
        return np.ascontiguousarray(
            w.T.reshape(2, P, P).transpose(1, 0, 2).reshape(P, 2 * P)
        ).astype(dt)

    import ml_dtypes
    wtT, wpT = pack(wt), pack(wp)
    wgT = pack(wg, ml_dtypes.bfloat16)
    wWT = np.ascontiguousarray(wW.T)                       # (128, 256)
    bWp = (wW @ bg + bW).astype(np.float32)                # fold bg into bW
    bWp = np.ascontiguousarray(bWp.reshape(2, P).T)        # (128, 2)
    shared = {
        "wtT": wtT, "wpT": wpT, "wgT": wgT, "wWT": wWT,
        "bt": bt.reshape(P, 1).copy(), "bp": bp.reshape(P, 1).copy(),
        "bWp": bWp,
    }
    return [{"x": np.ascontiguousarray(xf[b]),
             "xbf": np.ascontiguousarray(xf[b].astype(ml_dtypes.bfloat16)),
             **shared} for b in range(B)]


def kernel(x, wg, bg, wt, bt, wp, bp, wW, bW):
    from concourse.bass_utils import run_bass_kernel_spmd

    B, C_, H, W = np.asarray(x).shape
    in_maps = make_in_maps(x, wg, bg, wt, bt, wp, bp, wW, bW)
    nc = _built()
    res = run_bass_kernel_spmd(nc, in_maps, core_ids=list(range(B)))
    out = np.stack([res.results[b]["out"] for b in range(B)])
    return out.reshape(B, C_, H, W).astype(np.float32)


# revision 17
# speedup vs baseline: 1.0459x; 1.0001x over previous
"""NonLocalBlock (single-head attention, N=HW=4096, d=128) on 8 trn2 cores.

Sharding: data-parallel over batch (B=8) — one batch element per NeuronCore.
Per core, the whole block runs out of SBUF:

  xf (256, 4096) -> theta_T = wt@xf + bt      (128, N)   [PE + DVE bias]
                    phi     = wp@xf + bp      (128, N)   [PE + DVE bias]
                    g0      = (wg@xf)^T       (N, 128)   [PE bf16; bf16 x
                                                          via SWDGE cast-DMA]
  S^T[m, n] = sum_i phi[i,m] * theta_T[i,n]   (keys m on partitions)
  expS = exp(S^T - 40)                         [ACT]
  sums[n] = sum_m expS[m, n]                   [DVE/GP partials + PE fold]
  yu[o, n] = sum_m g0[m,o] expS[m,n]           (unnormalized)
  o[c,n] = (wW @ yu)[c,n] / sums[n] + bW'[c]
  out = o + xf   — via DMA: xf pre-stored to out, o added on top with an
                   SWDGE accumulate-store (CCE per-element add), so the
                   residual costs no compute-engine time.

Softmax runs without a per-row max: scores are ~N(0, 128) with empirical
|S| < ~91, so exp(S - 40) (a global shift — softmax-invariant) stays
inside fp32 range. The normalization is commuted past the wW matmul
(divide after, per-column), so the PSUM->SBUF copy of yu does not wait
on the reciprocal and the next q's matmuls start immediately.

Engine budget per core: PE ~136us (S/y matmuls), ACT ~134us (exp only),
DVE ~120us (bias adds, sum partials, epilogue), GpSimd ~50us (4 sum
chunks per q + SWDGE descriptor generation). Matmuls use float32r
(fp22, 1 PE pass); attention probabilities and g are bf16.
"""

import numpy as np
from contextlib import ExitStack

import concourse.bass as bass
import concourse.mybir as mybir
import concourse.tile as tile
from concourse import bacc

P = 128          # partitions / inter channels
C = 256          # input channels
F32 = mybir.dt.float32
F32R = mybir.dt.float32r
AF = mybir.ActivationFunctionType
ALU = mybir.AluOpType
BF16 = mybir.dt.bfloat16
CSHIFT = 40.0    # global score shift before exp (softmax-invariant)

B_FULL = 8
H_FULL = 64
W_FULL = 64
N_FULL = H_FULL * W_FULL

def build_nc(N=N_FULL, NQ=1024):
    """Build the single-core Bass module (SPMD: same NEFF on all 8 cores)."""
    assert N % 512 == 0 and NQ % 512 == 0 and N % NQ == 0
    MC = N // P                   # number of 128-row key chunks
    NB = NQ // 512                # 512-wide matmul blocks per quarter
    NQn = N // NQ                 # query quarters
    NBLK = N // 512               # 512-col x blocks

    nc = bacc.Bacc("TRN2", target_bir_lowering=False, debug=False)

    x_d = nc.dram_tensor("x", [C, N], F32R, kind="ExternalInput").ap()
    xbf_d = nc.dram_tensor("xbf", [C, N], BF16, kind="ExternalInput").ap()
    # weights host-packed to partition-major [128, 2*128] so DMAs are
    # trivially contiguous (one descriptor per partition)
    wtT_d = nc.dram_tensor("wtT", [P, 2 * P], F32R, kind="ExternalInput").ap()
    wpT_d = nc.dram_tensor("wpT", [P, 2 * P], F32R, kind="ExternalInput").ap()
    wgT_d = nc.dram_tensor("wgT", [P, 2 * P], BF16, kind="ExternalInput").ap()
    wWT_d = nc.dram_tensor("wWT", [P, C], F32R, kind="ExternalInput").ap()
    bt_d = nc.dram_tensor("bt", [P, 1], F32, kind="ExternalInput").ap()
    bp_d = nc.dram_tensor("bp", [P, 1], F32, kind="ExternalInput").ap()
    bWp_d = nc.dram_tensor("bWp", [P, 2], F32, kind="ExternalInput").ap()
    out_d = nc.dram_tensor("out", [C, N], F32R, kind="ExternalOutput").ap()

    x_v = x_d.rearrange("(k p) n -> k p n", p=P)
    xbf_v = xbf_d.rearrange("(k p) n -> k p n", p=P)
    out_v = out_d.rearrange("(k p) n -> k p n", p=P)

    with tile.TileContext(nc) as tc, ExitStack() as ctx:
        const = ctx.enter_context(tc.tile_pool(name="const", bufs=1))
        big = ctx.enter_context(tc.tile_pool(name="big", bufs=1))
        work = ctx.enter_context(tc.tile_pool(name="work", bufs=3))
        ps = ctx.enter_context(tc.tile_pool(name="ps", bufs=2, space="PSUM"))
        psy = ctx.enter_context(tc.tile_pool(name="psy", bufs=1, space="PSUM"))

        # ---- constant loads ----
        wtT_sb = const.tile([P, 2, P], F32R, name="wtT_sb")
        wpT_sb = const.tile([P, 2, P], F32R, name="wpT_sb")
        wgT_sb = const.tile([P, 2, P], BF16, name="wgT_sb")
        wWT_sb = const.tile([P, C], F32R, name="wWT_sb")
        bt_sb = const.tile([P, 1], F32, name="bt_sb")
        bp_sb = const.tile([P, 1], F32, name="bp_sb")
        bWp_sb = const.tile([P, 2], F32, name="bWp_sb")
        ones_sb = const.tile([P, P], BF16, name="ones_sb")
        cshift_sb = const.tile([P, 1], F32, name="cshift_sb")
        nc.vector.memset(cshift_sb[:], -CSHIFT)

        nc.sync.dma_start(wtT_sb[:], wtT_d.rearrange("p (k i) -> p k i", k=2))
        nc.sync.dma_start(wpT_sb[:], wpT_d.rearrange("p (k i) -> p k i", k=2))
        nc.sync.dma_start(wgT_sb[:], wgT_d.rearrange("p (k i) -> p k i", k=2))
        nc.sync.dma_start(wWT_sb[:], wWT_d)
        nc.sync.dma_start(bt_sb[:], bt_d)
        nc.sync.dma_start(bp_sb[:], bp_d)
        nc.sync.dma_start(bWp_sb[:], bWp_d)
        nc.vector.memset(ones_sb[:], 1.0)

        # ~10us of junk matmuls on memset data, sized to span the DMA
        # launch + first-x-block window (~15us): the PE crosses the HAM
        # activity threshold while waiting for data, so the first real
        # projection matmuls and the S->exp chain run at 2.4GHz, not
        # 1.2GHz. The tile holds one "s" slot until it finishes (~17us);
        # the first S matmul only needs the other slot (~16us).
        warm_ps = ps.tile([P, P], F32, tag="s", name="warm_ps")
        for _ in range(150):
            nc.tensor.matmul(warm_ps[:], ones_sb[:], ones_sb[:],
                             start=True, stop=True, skip_group_check=True)


        x_sb = big.tile([P, 2, N], F32R, name="x_sb")
        xbf_sb = big.tile([P, 2, N], BF16, name="xbf_sb")
        th_sb = big.tile([P, N], F32R, name="th_sb")   # theta^T (i, n)
        ph_sb = big.tile([P, N], F32R, name="ph_sb")   # phi (i, m)
        g_sb = big.tile([P, MC, P], BF16, name="g_sb")  # g0 (m_in, m_chunk, o)

        # ---- x load interleaved with th/ph projections, block by block.
        # xbf loads and the g matmuls are deferred behind the whole x
        # stream: x completes ~25% sooner (per-queue DMA bandwidth is the
        # startup limit), the th/ph proj-slot rotation never waits on
        # xbf, and g chunks still land well ahead of their y-matmul
        # consumers (one chunk per ~1.1us exp step).
        for blk in range(NBLK):
            sl = slice(blk * 512, (blk + 1) * 512)
            for k in range(2):
                nc.sync.dma_start(x_sb[:, k, sl], x_v[k, :, sl])

            th_ps = ps.tile([P, 512], F32, tag="proj", name="th_ps")
            nc.tensor.matmul(th_ps[:], wtT_sb[:, 0], x_sb[:, 0, sl],
                             start=True, stop=False)
            nc.tensor.matmul(th_ps[:], wtT_sb[:, 1], x_sb[:, 1, sl],
                             start=False, stop=True)
            nc.vector.tensor_scalar_add(th_sb[:, sl], th_ps[:], bt_sb[:, 0:1])

            ph_ps = ps.tile([P, 512], F32, tag="proj", name="ph_ps")
            nc.tensor.matmul(ph_ps[:], wpT_sb[:, 0], x_sb[:, 0, sl],
                             start=True, stop=False)
            nc.tensor.matmul(ph_ps[:], wpT_sb[:, 1], x_sb[:, 1, sl],
                             start=False, stop=True)
            nc.vector.tensor_scalar_add(ph_sb[:, sl], ph_ps[:], bp_sb[:, 0:1])

        for blk in range(NBLK):
            sl = slice(blk * 512, (blk + 1) * 512)
            for k in range(2):
                nc.sync.dma_start(xbf_sb[:, k, sl], xbf_v[k, :, sl])
            # g chunks for this block (bf16: full-rate 128-col matmuls)
            for j in range(4):
                mc = blk * 4 + j
                msl = slice(mc * P, (mc + 1) * P)
                g_ps = ps.tile([P, P], F32, tag="proj", name="g_ps")
                nc.tensor.matmul(g_ps[:], xbf_sb[:, 0, msl], wgT_sb[:, 0],
                                 start=True, stop=False)
                nc.tensor.matmul(g_ps[:], xbf_sb[:, 1, msl], wgT_sb[:, 1],
                                 start=False, stop=True)
                nc.vector.tensor_copy(g_sb[:, mc], g_ps[:])

        # residual: pre-store x to out for the first 3 quarters (their o
        # is added on top with SWDGE accumulate-stores); the last quarter
        # adds x on DVE and does a plain store to shorten the tail
        NPRE = N - NQ
        for k in range(2):
            nc.sync.dma_start(out_v[k, :, 0:NPRE], x_sb[:, k, 0:NPRE])

        # ---- attention main loop ----
        for q in range(NQn):
            qsl = slice(q * NQ, (q + 1) * NQ)
            y_ps = psy.tile([P, NQ], F32, tag="y", name="y_ps")
            acc = None             # DVE accumulator

            for mc in range(MC):
                msl = slice(mc * P, (mc + 1) * P)
                s_ps = ps.tile([P, NQ], F32, tag="s", name="s_ps")
                for b in range(NB):
                    bsl = slice(b * 512, (b + 1) * 512)
                    nc.tensor.matmul(
                        s_ps[:, bsl], ph_sb[:, msl],
                        th_sb[:, q * NQ + b * 512: q * NQ + (b + 1) * 512],
                        start=True, stop=True)
                exp_sb = work.tile([P, NQ], BF16, tag="exp", bufs=8,
                                   name="exp_sb")
                nc.scalar.activation(exp_sb[:], s_ps[:], AF.Exp,
                                     bias=cshift_sb[:, 0:1])

                for b in range(NB):
                    bsl = slice(b * 512, (b + 1) * 512)
                    nc.tensor.matmul(
                        y_ps[:, bsl], g_sb[:, mc], exp_sb[:, bsl],
                        start=(mc == 0), stop=(mc == MC - 1),
                        skip_group_check=True)

                # column-sum partials on DVE, single accumulator
                # (tensor_tensor runs at 2x on bf16; GpSimd stays idle —
                # it shares an SBUF port with DVE and contention costs
                # more than its offload saves; the 692ns add keeps pace
                # with the 1114ns exp so the chain never falls behind)
                if acc is None:
                    acc = work.tile([P, NQ], BF16, tag="acc", bufs=1,
                                    name="acc_sb")
                    nc.vector.tensor_copy(acc[:], exp_sb[:])
                else:
                    nc.vector.tensor_add(acc[:], acc[:], exp_sb[:])

            # unnormalized y out of PSUM immediately (frees y_ps for q+1;
            # does NOT wait on the sum/reciprocal path)
            yt_sb = work.tile([P, NQ], F32R, tag="yt", bufs=2, name="yt_sb")
            nc.vector.tensor_copy(yt_sb[:], y_ps[:])

            # Whole epilogue runs on 1-bank "proj" PSUM tiles (idle during
            # attention) in 512-col halves, so neither the "s" slots (S
            # prefetch) nor the "y" slot (next q's accumulation) is ever
            # held by epilogue work.
            recip_sb = work.tile([P, NQ], F32, tag="recip", bufs=2,
                                 name="recip_sb")
            for b in range(NB):
                bsl = slice(b * 512, (b + 1) * 512)
                sum_ps = ps.tile([P, 512], F32, tag="proj", name="sum_ps")
                nc.tensor.matmul(sum_ps[:], ones_sb[:], acc[:, bsl],
                                 start=True, stop=True,
                                 skip_group_check=True)
                nc.vector.reciprocal_approx_fast(recip_sb[:, bsl], sum_ps[:])

            # o = (wW @ yu) * recip + bW'; the +x rides the accumulate-
            # store except on the last q, where a DVE add + plain store in
            # 512-col pieces shortens the kernel tail.
            last = (q == NQn - 1)
            for h in range(2):
                o_sb = work.tile([P, NQ], F32R, tag="o", bufs=4, name="o_sb")
                for b in range(NB):
                    bsl = slice(b * 512, (b + 1) * 512)
                    wy_ps = ps.tile([P, 512], F32, tag="proj", name="wy_ps")
                    nc.tensor.matmul(
                        wy_ps[:], wWT_sb[:, h * P:(h + 1) * P],
                        yt_sb[:, bsl], start=True, stop=True)
                    nc.vector.tensor_mul(o_sb[:, bsl], wy_ps[:],
                                         recip_sb[:, bsl])
                    nc.vector.tensor_scalar_add(o_sb[:, bsl], o_sb[:, bsl],
                                                bWp_sb[:, h:h + 1])
                    if last:
                        nc.vector.tensor_add(o_sb[:, bsl], o_sb[:, bsl],
                                             x_sb[:, h, q * NQ + b * 512:
                                                  q * NQ + (b + 1) * 512])
                        nc.sync.dma_start(
                            out_v[h, :, q * NQ + b * 512:
                                  q * NQ + (b + 1) * 512], o_sb[:, bsl])
                if not last:
                    nc.gpsimd.dma_start(out_v[h, :, qsl], o_sb[:],
                                        accum_op=ALU.add)

    nc.compile()
    return nc


_CACHE = {}


def _built(key=(N_FULL, 1024)):
    if key not in _CACHE:
        _CACHE[key] = build_nc(*key)
    return _CACHE[key]


def make_in_maps(x, wg, bg, wt, bt, wp, bp, wW, bW):
    """Host-side prep: per-core input dicts (core b <- batch b)."""
    x = np.asarray(x, np.float32)
    B, C_, H, W = x.shape
    N = H * W
    xf = np.ascontiguousarray(x.reshape(B, C_, N))
    wg, bg, wt, bt, wp, bp, wW, bW = [
        np.asarray(a, np.float32) for a in (wg, bg, wt, bt, wp, bp, wW, bW)]
    def pack(w, dt=np.float32):  # (128, C) conv weight -> partition-major lhsT
        return np.ascontiguousarray(
            w.T.reshape(2, P, P).transpose(1, 0, 2).reshape(P, 2 * P)
        ).astype(dt)

    import ml_dtypes
    wtT, wpT = pack(wt), pack(wp)
    wgT = pack(wg, ml_dtypes.bfloat16)
    wWT = np.ascontiguousarray(wW.T)                       # (128, 256)
    bWp = (wW @ bg + bW).astype(np.float32)                # fold bg into bW
    bWp = np.ascontiguousarray(bWp.reshape(2, P).T)        # (128, 2)
    shared = {
        "wtT": wtT, "wpT": wpT, "wgT": wgT, "wWT": wWT,
        "bt": bt.reshape(P, 1).copy(), "bp": bp.reshape(P, 1).copy(),
        "bWp": bWp,
    }
    return [{"x": np.ascontiguousarray(xf[b]),
             "xbf": np.ascontiguousarray(xf[b].astype(ml_dtypes.bfloat16)),
             **shared} for b in range(B)]


def kernel(x, wg, bg, wt, bt, wp, bp, wW, bW):
    from concourse.bass_utils import run_bass_kernel_spmd

    B, C_, H, W = np.asarray(x).shape
    in_maps = make_in_maps(x, wg, bg, wt, bt, wp, bp, wW, bW)
    nc = _built()
    res = run_bass_kernel_spmd(nc, in_maps, core_ids=list(range(B)))
    out = np.stack([res.results[b]["out"] for b in range(B)])
    return out.reshape(B, C_, H, W).astype(np.float32)


# revision 18
# speedup vs baseline: 1.0508x; 1.0047x over previous
"""NonLocalBlock (single-head attention, N=HW=4096, d=128) on 8 trn2 cores.

Sharding: data-parallel over batch (B=8) — one batch element per NeuronCore.
Per core, the whole block runs out of SBUF:

  xf (256, 4096) -> theta_T = wt@xf + bt      (128, N)   [PE + DVE bias]
                    phi     = wp@xf + bp      (128, N)   [PE + DVE bias]
                    g0      = (wg@xf)^T       (N, 128)   [PE bf16; bf16 x
                                                          via SWDGE cast-DMA]
  S^T[m, n] = sum_i phi[i,m] * theta_T[i,n]   (keys m on partitions)
  expS = exp(S^T - 40)                         [ACT]
  sums[n] = sum_m expS[m, n]                   [DVE/GP partials + PE fold]
  yu[o, n] = sum_m g0[m,o] expS[m,n]           (unnormalized)
  o[c,n] = (wW @ yu)[c,n] / sums[n] + bW'[c]
  out = o + xf   — via DMA: xf pre-stored to out, o added on top with an
                   SWDGE accumulate-store (CCE per-element add), so the
                   residual costs no compute-engine time.

Softmax runs without a per-row max: scores are ~N(0, 128) with empirical
|S| < ~91, so exp(S - 40) (a global shift — softmax-invariant) stays
inside fp32 range. The normalization is commuted past the wW matmul
(divide after, per-column), so the PSUM->SBUF copy of yu does not wait
on the reciprocal and the next q's matmuls start immediately.

Engine budget per core: PE ~136us (S/y matmuls), ACT ~134us (exp only),
DVE ~120us (bias adds, sum partials, epilogue), GpSimd ~50us (4 sum
chunks per q + SWDGE descriptor generation). Matmuls use float32r
(fp22, 1 PE pass); attention probabilities and g are bf16.
"""

import numpy as np
from contextlib import ExitStack

import concourse.bass as bass
import concourse.mybir as mybir
import concourse.tile as tile
from concourse import bacc

P = 128          # partitions / inter channels
C = 256          # input channels
F32 = mybir.dt.float32
F32R = mybir.dt.float32r
AF = mybir.ActivationFunctionType
ALU = mybir.AluOpType
BF16 = mybir.dt.bfloat16
CSHIFT = 40.0    # global score shift before exp (softmax-invariant)

B_FULL = 8
H_FULL = 64
W_FULL = 64
N_FULL = H_FULL * W_FULL

def build_nc(N=N_FULL, NQ=1024):
    """Build the single-core Bass module (SPMD: same NEFF on all 8 cores)."""
    assert N % 512 == 0 and NQ % 512 == 0 and N % NQ == 0
    MC = N // P                   # number of 128-row key chunks
    NB = NQ // 512                # 512-wide matmul blocks per quarter
    NQn = N // NQ                 # query quarters
    NBLK = N // 512               # 512-col x blocks

    nc = bacc.Bacc("TRN2", target_bir_lowering=False, debug=False)

    x_d = nc.dram_tensor("x", [C, N], F32R, kind="ExternalInput").ap()
    xbf_d = nc.dram_tensor("xbf", [C, N], BF16, kind="ExternalInput").ap()
    # weights host-packed to partition-major [128, 2*128] so DMAs are
    # trivially contiguous (one descriptor per partition)
    wtT_d = nc.dram_tensor("wtT", [P, 2 * P], F32R, kind="ExternalInput").ap()
    wpT_d = nc.dram_tensor("wpT", [P, 2 * P], F32R, kind="ExternalInput").ap()
    wgT_d = nc.dram_tensor("wgT", [P, 2 * P], BF16, kind="ExternalInput").ap()
    wWT_d = nc.dram_tensor("wWT", [P, C], F32R, kind="ExternalInput").ap()
    bt_d = nc.dram_tensor("bt", [P, 1], F32, kind="ExternalInput").ap()
    bp_d = nc.dram_tensor("bp", [P, 1], F32, kind="ExternalInput").ap()
    bWp_d = nc.dram_tensor("bWp", [P, 2], F32, kind="ExternalInput").ap()
    out_d = nc.dram_tensor("out", [C, N], F32R, kind="ExternalOutput").ap()

    x_v = x_d.rearrange("(k p) n -> k p n", p=P)
    xbf_v = xbf_d.rearrange("(k p) n -> k p n", p=P)
    out_v = out_d.rearrange("(k p) n -> k p n", p=P)

    with tile.TileContext(nc) as tc, ExitStack() as ctx:
        const = ctx.enter_context(tc.tile_pool(name="const", bufs=1))
        big = ctx.enter_context(tc.tile_pool(name="big", bufs=1))
        work = ctx.enter_context(tc.tile_pool(name="work", bufs=3))
        ps = ctx.enter_context(tc.tile_pool(name="ps", bufs=2, space="PSUM"))
        psy = ctx.enter_context(tc.tile_pool(name="psy", bufs=1, space="PSUM"))

        # ---- constant loads ----
        wtT_sb = const.tile([P, 2, P], F32R, name="wtT_sb")
        wpT_sb = const.tile([P, 2, P], F32R, name="wpT_sb")
        wgT_sb = const.tile([P, 2, P], BF16, name="wgT_sb")
        wWT_sb = const.tile([P, C], F32R, name="wWT_sb")
        bt_sb = const.tile([P, 1], F32, name="bt_sb")
        bp_sb = const.tile([P, 1], F32, name="bp_sb")
        bWp_sb = const.tile([P, 2], F32, name="bWp_sb")
        ones_sb = const.tile([P, P], BF16, name="ones_sb")
        cshift_sb = const.tile([P, 1], F32, name="cshift_sb")
        nc.vector.memset(cshift_sb[:], -CSHIFT)

        nc.sync.dma_start(wtT_sb[:], wtT_d.rearrange("p (k i) -> p k i", k=2))
        nc.sync.dma_start(wpT_sb[:], wpT_d.rearrange("p (k i) -> p k i", k=2))
        nc.sync.dma_start(wgT_sb[:], wgT_d.rearrange("p (k i) -> p k i", k=2))
        nc.sync.dma_start(wWT_sb[:], wWT_d)
        nc.sync.dma_start(bt_sb[:], bt_d)
        nc.sync.dma_start(bp_sb[:], bp_d)
        nc.sync.dma_start(bWp_sb[:], bWp_d)
        nc.vector.memset(ones_sb[:], 1.0)

        # ~10us of junk matmuls on memset data, sized to span the DMA
        # launch + first-x-block window (~15us): the PE crosses the HAM
        # activity threshold while waiting for data, so the first real
        # projection matmuls and the S->exp chain run at 2.4GHz, not
        # 1.2GHz. The tile holds one "s" slot until it finishes (~17us);
        # the first S matmul only needs the other slot (~16us).
        warm_ps = ps.tile([P, P], F32, tag="s", name="warm_ps")
        for _ in range(150):
            nc.tensor.matmul(warm_ps[:], ones_sb[:], ones_sb[:],
                             start=True, stop=True, skip_group_check=True)


        x_sb = big.tile([P, 2, N], F32R, name="x_sb")
        xbf_sb = big.tile([P, 2, N], BF16, name="xbf_sb")
        th_sb = big.tile([P, N], F32R, name="th_sb")   # theta^T (i, n)
        ph_sb = big.tile([P, N], F32R, name="ph_sb")   # phi (i, m)
        g_sb = big.tile([P, MC, P], BF16, name="g_sb")  # g0 (m_in, m_chunk, o)

        # ---- x load interleaved with th/ph projections, block by block.
        # xbf loads and the g matmuls are deferred behind the whole x
        # stream: x completes ~25% sooner (per-queue DMA bandwidth is the
        # startup limit), the th/ph proj-slot rotation never waits on
        # xbf, and g chunks still land well ahead of their y-matmul
        # consumers (one chunk per ~1.1us exp step).
        for blk in range(NBLK):
            sl = slice(blk * 512, (blk + 1) * 512)
            for k in range(2):
                nc.sync.dma_start(x_sb[:, k, sl], x_v[k, :, sl])

            th_ps = ps.tile([P, 512], F32, tag="proj", name="th_ps")
            nc.tensor.matmul(th_ps[:], wtT_sb[:, 0], x_sb[:, 0, sl],
                             start=True, stop=False)
            nc.tensor.matmul(th_ps[:], wtT_sb[:, 1], x_sb[:, 1, sl],
                             start=False, stop=True)
            nc.vector.tensor_scalar_add(th_sb[:, sl], th_ps[:], bt_sb[:, 0:1])

            ph_ps = ps.tile([P, 512], F32, tag="proj", name="ph_ps")
            nc.tensor.matmul(ph_ps[:], wpT_sb[:, 0], x_sb[:, 0, sl],
                             start=True, stop=False)
            nc.tensor.matmul(ph_ps[:], wpT_sb[:, 1], x_sb[:, 1, sl],
                             start=False, stop=True)
            nc.vector.tensor_scalar_add(ph_sb[:, sl], ph_ps[:], bp_sb[:, 0:1])

        for blk in range(NBLK):
            sl = slice(blk * 512, (blk + 1) * 512)
            for k in range(2):
                nc.sync.dma_start(xbf_sb[:, k, sl], xbf_v[k, :, sl])
            # g chunks for this block (bf16: full-rate 128-col matmuls)
            for j in range(4):
                mc = blk * 4 + j
                msl = slice(mc * P, (mc + 1) * P)
                g_ps = ps.tile([P, P], F32, tag="proj", name="g_ps")
                nc.tensor.matmul(g_ps[:], xbf_sb[:, 0, msl], wgT_sb[:, 0],
                                 start=True, stop=False)
                nc.tensor.matmul(g_ps[:], xbf_sb[:, 1, msl], wgT_sb[:, 1],
                                 start=False, stop=True)
                nc.vector.tensor_copy(g_sb[:, mc], g_ps[:])

        # residual: pre-store x to out for the first 3 quarters (their o
        # is added on top with SWDGE accumulate-stores); the last quarter
        # adds x on DVE and does a plain store to shorten the tail
        NPRE = N - NQ
        for k in range(2):
            nc.sync.dma_start(out_v[k, :, 0:NPRE], x_sb[:, k, 0:NPRE])

        # ---- attention main loop ----
        for q in range(NQn):
            qsl = slice(q * NQ, (q + 1) * NQ)
            y_ps = psy.tile([P, NQ], F32, tag="y", name="y_ps")
            acc = None             # DVE accumulator

            for mc in range(MC):
                msl = slice(mc * P, (mc + 1) * P)
                s_ps = ps.tile([P, NQ], F32, tag="s", name="s_ps")
                for b in range(NB):
                    bsl = slice(b * 512, (b + 1) * 512)
                    nc.tensor.matmul(
                        s_ps[:, bsl], ph_sb[:, msl],
                        th_sb[:, q * NQ + b * 512: q * NQ + (b + 1) * 512],
                        start=True, stop=True)
                exp_sb = work.tile([P, NQ], BF16, tag="exp", bufs=10,
                                   name="exp_sb")
                nc.scalar.activation(exp_sb[:], s_ps[:], AF.Exp,
                                     bias=cshift_sb[:, 0:1])

                for b in range(NB):
                    bsl = slice(b * 512, (b + 1) * 512)
                    nc.tensor.matmul(
                        y_ps[:, bsl], g_sb[:, mc], exp_sb[:, bsl],
                        start=(mc == 0), stop=(mc == MC - 1),
                        skip_group_check=True)

                # column-sum partials on DVE, single accumulator
                # (tensor_tensor runs at 2x on bf16; GpSimd stays idle —
                # it shares an SBUF port with DVE and contention costs
                # more than its offload saves; the 692ns add keeps pace
                # with the 1114ns exp so the chain never falls behind)
                if acc is None:
                    acc = work.tile([P, NQ], BF16, tag="acc", bufs=1,
                                    name="acc_sb")
                    nc.vector.tensor_copy(acc[:], exp_sb[:])
                else:
                    nc.vector.tensor_add(acc[:], acc[:], exp_sb[:])

            # unnormalized y out of PSUM immediately (frees y_ps for q+1;
            # does NOT wait on the sum/reciprocal path)
            yt_sb = work.tile([P, NQ], F32R, tag="yt", bufs=3, name="yt_sb")
            if q == NQn - 1:
                nc.scalar.activation(yt_sb[:], y_ps[:], AF.Identity)
            else:
                nc.vector.tensor_copy(yt_sb[:], y_ps[:])

            # Whole epilogue runs on 1-bank "proj" PSUM tiles (idle during
            # attention) in 512-col halves, so neither the "s" slots (S
            # prefetch) nor the "y" slot (next q's accumulation) is ever
            # held by epilogue work.
            recip_sb = work.tile([P, NQ], F32, tag="recip", bufs=3,
                                 name="recip_sb")
            for b in range(NB):
                bsl = slice(b * 512, (b + 1) * 512)
                sum_ps = ps.tile([P, 512], F32, tag="proj", name="sum_ps")
                nc.tensor.matmul(sum_ps[:], ones_sb[:], acc[:, bsl],
                                 start=True, stop=True,
                                 skip_group_check=True)
                nc.vector.reciprocal_approx_fast(recip_sb[:, bsl], sum_ps[:])

            # o = (wW @ yu) * recip + bW'; the +x rides the accumulate-
            # store except on the last q, where a DVE add + plain store in
            # 512-col pieces shortens the kernel tail.
            last = (q == NQn - 1)
            for h in range(2):
                o_sb = work.tile([P, NQ], F32R, tag="o", bufs=6, name="o_sb")
                for b in range(NB):
                    bsl = slice(b * 512, (b + 1) * 512)
                    wy_ps = ps.tile([P, 512], F32, tag="proj", name="wy_ps")
                    nc.tensor.matmul(
                        wy_ps[:], wWT_sb[:, h * P:(h + 1) * P],
                        yt_sb[:, bsl], start=True, stop=True)
                    nc.vector.tensor_mul(o_sb[:, bsl], wy_ps[:],
                                         recip_sb[:, bsl])
                    nc.vector.tensor_scalar_add(o_sb[:, bsl], o_sb[:, bsl],
                                                bWp_sb[:, h:h + 1])
                    if last:
                        nc.vector.tensor_add(o_sb[:, bsl], o_sb[:, bsl],
                                             x_sb[:, h, q * NQ + b * 512:
                                                  q * NQ + (b + 1) * 512])
                        nc.sync.dma_start(
                            out_v[h, :, q * NQ + b * 512:
                                  q * NQ + (b + 1) * 512], o_sb[:, bsl])
                if not last:
                    nc.gpsimd.dma_start(out_v[h, :, qsl], o_sb[:],
                                        accum_op=ALU.add)

    nc.compile()
    return nc


_CACHE = {}


def _built(key=(N_FULL, 1024)):
    if key not in _CACHE:
        _CACHE[key] = build_nc(*key)
    return _CACHE[key]


def make_in_maps(x, wg, bg, wt, bt, wp, bp, wW, bW):
    """Host-side prep: per-core input dicts (core b <- batch b)."""
    x = np.asarray(x, np.float32)
    B, C_, H, W = x.shape
    N = H * W
    xf = np.ascontiguousarray(x.reshape(B, C_, N))
    wg, bg, wt, bt, wp, bp, wW, bW = [
        np.asarray(a, np.float32) for a in (wg, bg, wt, bt, wp, bp, wW, bW)]
    def pack(w, dt=np.float32):  # (128, C) conv weight -> partition-major lhsT
        return np.ascontiguousarray(
            w.T.reshape(2, P, P).transpose(1, 0, 2).reshape(P, 2 * P)
        ).astype(dt)

    import ml_dtypes
    wtT, wpT = pack(wt), pack(wp)
    wgT = pack(wg, ml_dtypes.bfloat16)
    wWT = np.ascontiguousarray(wW.T)                       # (128, 256)
    bWp = (wW @ bg + bW).astype(np.float32)                # fold bg into bW
    bWp = np.ascontiguousarray(bWp.reshape(2, P).T)        # (128, 2)
    shared = {
        "wtT": wtT, "wpT": wpT, "wgT": wgT, "wWT": wWT,
        "bt": bt.reshape(P, 1).copy(), "bp": bp.reshape(P, 1).copy(),
        "bWp": bWp,
    }
    return [{"x": np.ascontiguousarray(xf[b]),
             "xbf": np.ascontiguousarray(xf[b].astype(ml_dtypes.bfloat16)),
             **shared} for b in range(B)]


def kernel(x, wg, bg, wt, bt, wp, bp, wW, bW):
    from concourse.bass_utils import run_bass_kernel_spmd

    B, C_, H, W = np.asarray(x).shape
    in_maps = make_in_maps(x, wg, bg, wt, bt, wp, bp, wW, bW)
    nc = _built()
    res = run_bass_kernel_spmd(nc, in_maps, core_ids=list(range(B)))
    out = np.stack([res.results[b]["out"] for b in range(B)])
    return out.reshape(B, C_, H, W).astype(np.float32)
